# revision 88
# baseline (speedup 1.0000x reference)
"""Trainium2 Bass kernel for a full transformer block (LN->MHA->LN->FFN).

Sharding: 4-way data-parallel over batch x 2-way tensor-parallel within
each pair. Phase 1 (LN1+QKV+attention+proj) splits heads across the pair
(each core: 8 of 16 heads, full T=2048 rows). The proj partial sums are
pairwise ReduceScatter'd (bf16), after which each core owns 256 rows of
each 512-row chunk and runs phase 2 (x2+LN2+FFN+out) on its own rows with
the FULL FFN weights - no second collective, and the final residual is
fused into phase 2 (no extra x / attn reads).

LayerNorm scale vectors are folded into Wq/Wk/Wv/W1 rows on the host.
Activation-table placement is patched so ln/exp/relu share one table set
(natural_log_exp_and_others), avoiding per-LayerNorm table reloads.

Self-contained: hardcodes the full-problem shapes; builds per-core input
slices on the host, runs one SPMD Bass program on 8 NeuronCores.
"""

import os
import numpy as np
import ml_dtypes

import concourse.bacc as bacc
import concourse.tile as tile
from concourse import mybir
from concourse.bass_utils import run_bass_kernel_spmd

STUB_CC = os.environ.get("STUB_CC") == "1"  # replace collectives with DMA copies (timing experiments only)
F32 = mybir.dt.float32
F32R = mybir.dt.float32r
BF16 = mybir.dt.bfloat16
FP8 = mybir.dt.float8e4
EPS = 1e-5


# ---- activation-table placement patch ----
# The default placement maps Exp/Relu to set "exp_and_others" and Ln to
# "natural_log", forcing a 1.3us table reload around every LayerNorm. All
# three live together in "natural_log_exp_and_others"; restrict them to
# that set so exactly one load is emitted.
_GAT_ORIG = None


def _patch_act_tables():
    global _GAT_ORIG
    if _GAT_ORIG is not None:
        return
    _GAT_ORIG = bacc.get_activation_tables

    combo = "natural_log_exp_and_others"
    only = {
        mybir.ActivationFunctionType.Ln,
        mybir.ActivationFunctionType.Exp,
        mybir.ActivationFunctionType.Relu,
        mybir.ActivationFunctionType.Identity,
        mybir.ActivationFunctionType.Copy,
    }

    def patched(arch):
        orig = _GAT_ORIG(arch)
        if combo not in orig:
            return orig
        return {
            k: (set(v) if k == combo else set(v) - only)
            for k, v in orig.items()
        }

    bacc.get_activation_tables = patched


class Cfg:
    def __init__(self, B, T, E, HPC, FH, n_cores):
        self.B, self.T, self.E, self.HPC, self.FH = B, T, E, HPC, FH
        self.n_cores = n_cores
        self.HS = 64
        self.D = HPC * self.HS          # local head dims (= cols of Wq slice)
        self.PAIRS = HPC // 2           # 128-col head-pair groups
        self.TT = T // 128              # t-tiles
        self.QCW = min(512, T)          # q-chunk width for attention
        self.TC = T // self.QCW        # q/t-chunks
        self.KTPQ = self.QCW // 128     # k-tiles per q-chunk block
        self.TPC = self.QCW // 128      # t-tiles per chunk
        self.EC = E // 128              # e-chunks
        self.NH = min(512, E)           # matmul N for E-wide outputs
        self.EH = E // self.NH          # n-halves of E
        self.OWN = self.QCW // 2        # own rows per chunk after RS
        self.OTPC = self.OWN // 128     # own t-tiles per chunk
        self.FC = FH // 128             # FFN hidden chunks (full hidden)
        self.scale = 1.0 / np.sqrt(E)


FULL = Cfg(B=4, T=2048, E=1024, HPC=8, FH=4096, n_cores=8)


def build_nc(cfg, repeats=1):
    _patch_act_tables()
    c = cfg
    nc = bacc.Bacc(
        "TRN2", target_bir_lowering=False, debug=False, num_devices=c.n_cores
    )
    pairs_rg = [[2 * i, 2 * i + 1] for i in range(c.n_cores // 2)]

    # ---- DRAM I/O ----
    # weight layouts are host-prepped so every DMA is [128, n] with
    # contiguous >=1KB lines (see make_in_maps)
    x_d = nc.dram_tensor("x", [c.T, c.E], F32, kind="ExternalInput")
    xo_d = nc.dram_tensor("xo", [c.T // 2, c.E], F32, kind="ExternalInput")
    # wq/wk/wv: fp8e4 (x256 / x256 / x64 host scaling, rescaled in the
    # PSUM->SBUF copies) for DoubleRow QKV matmuls
    wq_d = nc.dram_tensor("wq", [128, c.PAIRS * c.E], FP8, kind="ExternalInput")
    wk_d = nc.dram_tensor("wk", [128, c.PAIRS * c.E], FP8, kind="ExternalInput")
    wv_d = nc.dram_tensor("wv", [128, c.EC * c.D], FP8, kind="ExternalInput")
    wo_d = nc.dram_tensor("wo", [128, c.PAIRS * c.E], BF16, kind="ExternalInput")
    w1_d = nc.dram_tensor(
        "w1", [(c.FC // 2) * 128, c.EC * 256], BF16, kind="ExternalInput"
    )
    w2_d = nc.dram_tensor("w2", [128, c.FC * c.E], BF16, kind="ExternalInput")
    b1_d = nc.dram_tensor("b1", [128, c.FC], F32, kind="ExternalInput")
    bo_d = nc.dram_tensor("bor", [128, c.E], F32, kind="ExternalInput")
    b2_d = nc.dram_tensor("b2r", [128, c.E], F32, kind="ExternalInput")
    msk_d = nc.dram_tensor(
        "masks", [128, 2 * c.KTPQ * c.QCW], FP8, kind="ExternalInput"
    )
    id_d = nc.dram_tensor("ident", [128, 128], BF16, kind="ExternalInput")
    out_d = nc.dram_tensor("out", [c.T // 2, c.E], F32, kind="ExternalOutput")

    # ---- persistent SBUF ----
    # q/k stored fp8: q_st = q/4 (q is pre-scaled by E^-0.5 -> sigma
    # ~0.16), k_st = 8k (sigma ~5); scores come out 64x large, folded
    # into the exp's scale argument.
    qkT = nc.alloc_sbuf_tensor("qkT", [128, 2 * c.PAIRS * c.T], FP8).ap()

    def qT(p):
        return qkT[:, p * c.T:(p + 1) * c.T]

    def kT(p):
        return qkT[:, (c.PAIRS + p) * c.T:(c.PAIRS + p + 1) * c.T]

    aT_sb = nc.alloc_sbuf_tensor("aT_sb", [128, c.FC * c.OWN], BF16).ap()

    def aT(f):
        return aT_sb[:, f * c.OWN:(f + 1) * c.OWN]

    v_sb = nc.alloc_sbuf_tensor("v_sb", [128, c.TT * c.HPC * 65], BF16).ap()

    def v_aug(tt, h):
        o = (tt * c.HPC + h) * 65
        return v_sb[:, o:o + 65]

    NSLOTW = 3
    attT = nc.alloc_sbuf_tensor("attT", [128, NSLOTW * 2 * c.QCW], BF16).ap()

    ident = nc.alloc_sbuf_tensor("ident_sb", [128, 128], BF16).ap()
    masks = nc.alloc_sbuf_tensor("masks_sb", [128, 2 * c.KTPQ * c.QCW], FP8).ap()
    bo_r = nc.alloc_sbuf_tensor("bo_sb", [128, c.E], F32).ap()
    b2_r = nc.alloc_sbuf_tensor("b2_sb", [128, c.E], F32).ap()
    b1_sb = nc.alloc_sbuf_tensor("b1_sb", [128, c.FC], F32).ap()
    eps_sb = nc.alloc_sbuf_tensor("eps_sb", [128, 1], F32).ap()

    # ---- internal DRAM ----
    ar1_in = nc.dram_tensor("ar1_in", [c.T, c.E], BF16, kind="Internal")
    rs_out = nc.dram_tensor("rs_out", [c.T // 2, c.E], BF16, kind="Internal")

    with tile.TileContext(nc) as tc:
        with (
            tc.tile_pool(name="io", bufs=6) as io,
            tc.tile_pool(name="hT", bufs=2) as hpool,
            tc.tile_pool(name="x2p", bufs=4) as x2pool,
            tc.tile_pool(name="yTp", bufs=2) as ypool,
            tc.tile_pool(name="scr", bufs=3) as scr,
            tc.tile_pool(name="stat", bufs=2) as stat,
            tc.tile_pool(name="wqk", bufs=3) as wqk_pool,
            tc.tile_pool(name="w1p", bufs=3) as w1_pool,
            tc.tile_pool(name="w2p", bufs=4) as w2_pool,
            tc.tile_pool(name="wvp", bufs=1) as wv_pool,
            tc.tile_pool(name="wop", bufs=1) as wo_pool,
            tc.tile_pool(name="rcp", bufs=1) as rcp,
            tc.tile_pool(name="ps_w", bufs=2, space="PSUM") as ps_w,
            tc.tile_pool(name="ps_tp", bufs=1, space="PSUM") as ps_tp,
            tc.tile_pool(name="ps_acc", bufs=1, space="PSUM") as ps_acc,
            tc.tile_pool(name="ps_yps", bufs=1, space="PSUM") as ps_yps,
        ):
            # ---- consts (ident first: transposes need it ~4us in) ----
            nc.gpsimd.dma_start(ident[:], id_d[:])
            nc.gpsimd.dma_start(b1_sb[:], b1_d[:])
            nc.gpsimd.dma_start(masks[:], msk_d[:])
            nc.gpsimd.dma_start(bo_r[:], bo_d[:])
            nc.gpsimd.dma_start(b2_r[:], b2_d[:])
            nc.vector.memset(eps_sb[:], EPS)

            def layernorm_tile(xt):
                """xt: [128, E] f32 SBUF -> h [128, E] f32r tile.

                Scale weight is pre-folded into the consumer matmul weights.
                rsqrt(v) = exp(-0.5*ln(v)); ln/exp/relu share one ACT table.
                """
                ng = c.E // 512
                bst = stat.tile([128, 6 * ng], F32, tag="bst")
                bst3 = bst[:].rearrange("p (g s) -> p g s", g=ng)
                for g in range(ng):
                    nc.vector.bn_stats(
                        bst3[:, g:g + 1, :],
                        xt[:, g * 512:(g + 1) * 512].rearrange(
                            "p (g w) -> p g w", g=1
                        ),
                    )
                mv = stat.tile([128, 2], F32, tag="mv")
                nc.vector.bn_aggr(
                    mv[:], bst[:].rearrange("p (g s) -> p g s", g=ng)
                )
                mu = mv[:, 0:1]
                lnv = stat.tile([128, 1], F32, tag="lnv")
                nc.scalar.activation(
                    lnv[:], mv[:, 1:2], mybir.ActivationFunctionType.Ln,
                    bias=eps_sb[:],
                )
                rsig = stat.tile([128, 1], F32, tag="rsig")
                nc.scalar.activation(
                    rsig[:], lnv[:], mybir.ActivationFunctionType.Exp,
                    scale=-0.5,
                )
                h = scr.tile([128, c.E], BF16, tag="h")
                nc.vector.tensor_scalar(
                    h[:], xt[:], mu, rsig[:],
                    mybir.AluOpType.subtract, mybir.AluOpType.mult,
                )
                return h

            TG = 4  # transposes per psum tile

            def transpose_to(h, hTc, tt_loc, width):
                """h [128,E] f32r -> hTc e-chunk columns tt_loc (transposed).

                hTc layout: [128, EC * width]; block tt_loc covers columns
                [tt_loc*128, (tt_loc+1)*128) of each e-chunk.
                """
                dst3 = hTc.rearrange("p (e w) -> p e w", e=c.EC)[
                    :, :, tt_loc * 128:(tt_loc + 1) * 128
                ]
                for g0 in range(0, c.EC, TG):
                    tp = ps_tp.tile([128, TG * 128], BF16, tag="tp")
                    for i in range(TG):
                        e = g0 + i
                        nc.tensor.matmul(
                            tp[:, i * 128:(i + 1) * 128],
                            h[:, e * 128:(e + 1) * 128],
                            ident[:],
                            is_transpose=True, start=True, stop=True,
                        )
                    nc.vector.tensor_copy(
                        dst3[:, g0:g0 + TG, :],
                        tp[:].rearrange("p (g w) -> p g w", g=TG),
                    )

            slot_ctr = [0]

            def att_block(p, qc, yTc, mask_eng=None):
                """Attention for head pair p, q-chunk qc (kT/v ready).

                Even/odd head scores live in halves of one wide [128,1024]
                PSUM tile so exp and masking are single wide ops.
                """
                last = c.KTPQ * qc + c.KTPQ - 1
                q0 = qc * c.QCW
                W = c.QCW
                yps = ps_yps.tile([65, 2 * W], F32, tag="yps")
                pend = []

                def issue_av(kt, cq0, aw):
                    st, sp = kt == 0, kt == last
                    nc.tensor.matmul(
                        yps[:, cq0:W], v_aug(kt, 2 * p), aw[:, cq0:W],
                        start=st, stop=sp,
                    )
                    nc.tensor.matmul(
                        yps[:, W + cq0:], v_aug(kt, 2 * p + 1),
                        aw[:, W + cq0:],
                        start=st, stop=sp,
                    )

                for kt in range(last + 1):
                    j = kt - c.KTPQ * qc  # >=0: diagonal block stripe
                    # columns q < j*128 are fully masked: skip them entirely
                    cq0 = max(0, j) * 128
                    sw = ps_w.tile([128, 2 * W], F32, tag="w")
                    for hh in (0, 1):
                        off = hh * 64
                        nc.tensor.matmul(
                            sw[:, hh * W + cq0:(hh + 1) * W],
                            kT(p)[off:off + 64, kt * 128:(kt + 1) * 128],
                            qT(p)[off:off + 64, q0 + cq0:q0 + c.QCW],
                            start=True, stop=True,
                            tile_position=(off, 0),
                        )
                    s0 = (slot_ctr[0] % NSLOTW) * 2 * W
                    aw = attT[:, s0:s0 + 2 * W]
                    slot_ctr[0] += 1
                    # one exp over both halves (strided past skipped cols);
                    # 1/64 undoes the fp8 q/k storage scaling
                    nc.scalar.activation(
                        aw.rearrange("p (h w) -> p h w", h=2)[:, :, cq0:],
                        sw[:].rearrange("p (h w) -> p h w", h=2)[:, :, cq0:],
                        mybir.ActivationFunctionType.Exp,
                        scale=1.0 / 64.0,
                    )
                    if j >= 0:  # triangular mask on the surviving stripe
                        m2 = masks[:, 2 * j * W:2 * (j + 1) * W].rearrange(
                            "p (h w) -> p h w", h=2
                        )[:, :, cq0:]
                        a3 = aw.rearrange("p (h w) -> p h w", h=2)[:, :, cq0:]
                        (mask_eng or nc.gpsimd).tensor_mul(a3, a3, m2)
                    pend.append((kt, cq0, aw))
                    if len(pend) > 1:
                        issue_av(*pend.pop(0))
                while pend:
                    issue_av(*pend.pop(0))

                # normalize: yTc[p] rows = yps[0:64] * (1/yps[64])
                rc = rcp.tile([1, 2 * W], F32, tag="rc")
                nc.vector.reciprocal(rc[:], yps[64:65, :])
                rb = rcp.tile([64, 2 * W], F32, tag="rb")
                nc.gpsimd.partition_broadcast(rb[:], rc[:])
                for hh in (0, 1):
                    nc.vector.tensor_mul(
                        yTc[hh * 64:hh * 64 + 64, p * W:(p + 1) * W],
                        rb[:, hh * W:(hh + 1) * W],
                        yps[0:64, hh * W:(hh + 1) * W],
                    )

            def xload(tcc):
                """x tile loads for chunk tcc (emitted early, on SP)."""
                xts = []
                for tt_loc in range(c.TPC):
                    tt = tcc * c.TPC + tt_loc
                    xt = io.tile([128, c.E], F32, tag="io")
                    nc.sync.dma_start(xt[:], x_d[tt * 128:(tt + 1) * 128, :])
                    xts.append(xt)
                return xts

            def phase1a_ln(tcc, xts):
                """LN1 + transposes for chunk tcc -> hTc (fp8, the QKV
                DoubleRow moving/stationary operand)."""
                hTc = hpool.tile([128, c.EC * c.QCW], FP8, tag="hT")
                for tt_loc in range(c.TPC):
                    h = layernorm_tile(xts[tt_loc])
                    transpose_to(h, hTc[:, :], tt_loc, c.QCW)
                return hTc

            def phase1a_mm(tcc, hTc):
                """V + QK for chunk tcc; returns wot for phase1b."""
                r0 = tcc * c.QCW  # first row of chunk

                # weight prefetch for this chunk (ACT queue: idle here --
                # except at startup, where ACT must run chunk 0's LN)
                weng = nc.sync if tcc == 0 else nc.scalar
                wvt = wv_pool.tile([128, c.EC * c.D], FP8, tag="wv")
                weng.dma_start(wvt[:], wv_d[:])
                wot = wo_pool.tile([128, c.PAIRS * c.E], BF16, tag="wo")
                weng.dma_start(wot[:], wo_d[:])

                DRm = mybir.MatmulPerfMode.DoubleRow
                EPm = c.EC // 2

                # V for this chunk's t-tiles (fp8 DoubleRow over e-pairs)
                for tt_loc in range(c.TPC):
                    tt = tcc * c.TPC + tt_loc
                    vps = ps_acc.tile([128, c.D], F32, tag="acc")
                    for i in range(EPm):
                        lh = hTc[:, 2 * i * c.QCW:(2 * i + 2) * c.QCW]
                        nc.tensor.matmul(
                            vps[:],
                            lh.rearrange("p (j w) -> p j w", j=2)[
                                :, :, tt_loc * 128:(tt_loc + 1) * 128
                            ],
                            wvt[:, 2 * i * c.D:(2 * i + 2) * c.D]
                            .rearrange("p (j w) -> p j w", j=2),
                            start=(i == 0), stop=(i == EPm - 1),
                            perf_mode=DRm,
                        )
                    vdst = v_sb[
                        :, tt * c.HPC * 65:(tt + 1) * c.HPC * 65
                    ].rearrange("p (h w) -> p h w", w=65)
                    nc.vector.tensor_scalar_mul(
                        vdst[:, :, 0:64],
                        vps[:].rearrange("p (h w) -> p h w", w=64),
                        1.0 / 64.0,
                    )
                    nc.vector.memset(vdst[:, :, 64:65], 1.0)

                # Q/K for this chunk (fp8 DoubleRow over e-pairs)
                for p in range(c.PAIRS):
                    wqt = wqk_pool.tile([128, c.E], FP8, tag="wqk")
                    nc.sync.dma_start(wqt[:], wq_d[:, p * c.E:(p + 1) * c.E])
                    wkt = wqk_pool.tile([128, c.E], FP8, tag="wqk")
                    nc.sync.dma_start(wkt[:], wk_d[:, p * c.E:(p + 1) * c.E])
                    qk = ps_w.tile([128, 2 * c.QCW], F32, tag="w")
                    for half, wt in ((0, wqt), (1, wkt)):
                        for i in range(EPm):
                            nc.tensor.matmul(
                                qk[:, half * c.QCW:(half + 1) * c.QCW],
                                wt[:, 2 * i * 128:(2 * i + 2) * 128]
                                .rearrange("p (j m) -> p j m", j=2),
                                hTc[:, 2 * i * c.QCW:(2 * i + 2) * c.QCW]
                                .rearrange("p (j w) -> p j w", j=2),
                                start=(i == 0), stop=(i == EPm - 1),
                                perf_mode=DRm,
                            )
                    # q slice and k slice of qkT are PAIRS*T cols apart
                    dqk = qkT.rearrange(
                        "p (s w) -> p s w", w=c.PAIRS * c.T
                    )[:, :, p * c.T + r0:p * c.T + r0 + c.QCW]
                    nc.vector.tensor_scalar_mul(
                        dqk, qk[:].rearrange("p (s w) -> p s w", s=2),
                        1.0 / 32.0,
                    )

                return wot

            def phase1b(tcc, wot, mid=None, mid_p=1):
                """Attention + proj -> ar1_in chunk + ReduceScatter.

                `mid` is emitted after the first head pair so its DVE/PE
                work overlaps the remaining attention. For the last chunk
                the RS is split into two half-chunk pieces so the tail
                phase2 can start on the first half.
                """
                r0 = tcc * c.QCW
                yTc = ypool.tile([128, c.PAIRS * c.QCW], BF16, tag="yT")
                # chunk 1's attention overlaps RS(0), whose trigger holds
                # the gpsimd queue -- route its mask muls via DVE instead
                meng = nc.vector if tcc == 1 else None
                mid_out = None
                for p in range(c.PAIRS):
                    att_block(p, tcc, yTc[:, :], mask_eng=meng)
                    if p == mid_p and mid is not None:
                        mid_out = mid()

                for tt_loc in range(c.TPC):
                    tt = tcc * c.TPC + tt_loc
                    pt = io.tile([128, c.E], BF16, tag="iop")
                    pp = ps_w.tile([128, c.EH * c.NH], F32, tag="w")
                    for eh in range(c.EH):
                        for d in range(c.PAIRS):
                            nc.tensor.matmul(
                                pp[:, eh * c.NH:(eh + 1) * c.NH],
                                yTc[:, d * c.QCW + tt_loc * 128:][:, :128],
                                wot[:, d * c.E + eh * c.NH:][:, :c.NH],
                                start=(d == 0), stop=(d == c.PAIRS - 1),
                            )
                    nc.vector.tensor_copy(pt[:], pp[:])
                    nc.sync.dma_start(ar1_in[tt * 128:(tt + 1) * 128, :], pt[:])

                rows = slice(r0, r0 + c.QCW)
                orows = slice(tcc * c.OWN, (tcc + 1) * c.OWN)
                if c.n_cores == 1 or STUB_CC:  # timing stub
                    nc.gpsimd.dma_start(
                        rs_out[orows, :], ar1_in[r0:r0 + c.OWN, :]
                    )
                else:
                    nc.gpsimd.collective_compute(
                        "ReduceScatter", mybir.AluOpType.add,
                        replica_groups=pairs_rg,
                        ins=[ar1_in[rows, :]], outs=[rs_out[orows, :]],
                    )
                return mid_out

            def phase2a_pre(tcc):
                """Own rows of chunk tcc: x2 + LN2 (no transposes yet)."""
                x2s, h2s = [], []
                for tt_loc in range(c.OTPC):
                    r = tcc * c.OWN + tt_loc * 128
                    xt = io.tile([128, c.E], F32, tag="io")
                    nc.gpsimd.dma_start(xt[:], xo_d[r:r + 128, :])
                    at = io.tile([128, c.E], BF16, tag="iop")
                    nc.gpsimd.dma_start(at[:], rs_out[r:r + 128, :])
                    x2 = x2pool.tile([128, c.E], F32, tag="x2")
                    nc.gpsimd.tensor_add(x2[:], xt[:], at[:])
                    nc.gpsimd.tensor_add(x2[:], x2[:], bo_r[:])
                    h2 = layernorm_tile(x2)
                    x2s.append(x2)
                    h2s.append(h2)
                return x2s, h2s

            def phase2a_tp(h2s):
                """Transposes for phase2a (emitted mid-attention)."""
                h2T = hpool.tile([128, c.EC * c.OWN], BF16, tag="hT")
                for tt_loc in range(c.OTPC):
                    transpose_to(h2s[tt_loc], h2T[:, :], tt_loc, c.OWN)
                return h2T

            def w1_load(fgp):
                w1t = w1_pool.tile([128, c.EC * 256], BF16, tag="w1")
                nc.sync.dma_start(
                    w1t[:], w1_d[fgp * 128:(fgp + 1) * 128, :]
                )
                return w1t

            def w2_load(f):
                w2t = w2_pool.tile([128, c.E], BF16, tag="w2")
                nc.scalar.dma_start(w2t[:], w2_d[:, f * c.E:(f + 1) * c.E])
                return w2t

            def prefetch_p2():
                """First FFN weight tiles, loaded under the attention block."""
                return ([w1_load(fgp) for fgp in range(3)],
                        [w2_load(f) for f in range(2)])

            def phase2b(tcc, x2s, h2s, pre):
                """FFN + residual out for own rows of chunk tcc."""
                h2T = phase2a_tp(h2s)
                w1ts, w2ts = pre if pre else ([], [])
                for fgp in range(c.FC // 2):
                    w1t = w1ts[fgp] if fgp < len(w1ts) else w1_load(fgp)
                    for gi in range(2):
                        f = 2 * fgp + gi
                        ap_ = ps_acc.tile([128, c.OWN], F32, tag="acc")
                        for e in range(c.EC):
                            nc.tensor.matmul(
                                ap_[:],
                                w1t[:, e * 256 + gi * 128:][:, :128],
                                h2T[:, e * c.OWN:(e + 1) * c.OWN],
                                start=(e == 0), stop=(e == c.EC - 1),
                            )
                        nc.scalar.activation(
                            aT(f), ap_[:], mybir.ActivationFunctionType.Relu,
                            bias=b1_sb[:, f:f + 1],
                        )

                # FFN2: ff[tt] accumulates over all FC f-groups
                ffs = []
                for _ti in range(c.OTPC):
                    fftile = ps_w.tile([128, c.E], F32, tag="w")
                    ffs.append(fftile)
                for f in range(c.FC):
                    w2t = w2ts[f] if f < len(w2ts) else w2_load(f)
                    for tt_loc in range(c.OTPC):
                        for eh in range(c.EH):
                            nc.tensor.matmul(
                                ffs[tt_loc][:, eh * c.NH:(eh + 1) * c.NH],
                                aT(f)[:, tt_loc * 128:(tt_loc + 1) * 128],
                                w2t[:, eh * c.NH:(eh + 1) * c.NH],
                                start=(f == 0), stop=(f == c.FC - 1),
                            )
                return ffs

            def p2_out(tcc, x2s, ffs):
                """out rows = x2 + ff + b2 (emitted after next LN1)."""
                for tt_loc in range(c.OTPC):
                    r = tcc * c.OWN + tt_loc * 128
                    ot = io.tile([128, c.E], F32, tag="io")
                    nc.vector.tensor_add(ot[:], x2s[tt_loc][:], ffs[tt_loc][:])
                    nc.gpsimd.tensor_add(ot[:], ot[:], b2_r[:])
                    nc.sync.dma_start(out_d[r:r + 128, :], ot[:])

            # ---- software-pipelined emission over chunks ----
            # phase2a_pre(tc-1) emits under phase1a_mm(tc) (its rs_out
            # loads wait the RS semaphore on the gpsimd queue); the FFN
            # weights prefetch under the attention block; phase2b (which
            # starts with the h2 transposes) fills the PE queue after
            # phase1b(tc); the next chunk's x loads + LN1 emit before the
            # out-adds so DVE feeds PE first. The last chunk's phase2a_pre
            # emits directly after its RS trigger (nothing left to block).
            for _rep in range(repeats):
                hT_cur = phase1a_ln(0, xload(0))
                for tcc in range(c.TC):
                    wot = phase1a_mm(tcc, hT_cur)
                    if tcc >= 1:
                        mid = lambda: (phase2a_pre(tcc - 1), prefetch_p2())
                    else:
                        mid = None
                    mid_out = phase1b(
                        tcc, wot, mid=mid, mid_p=3 if tcc == 1 else 1
                    )
                    if tcc >= 1:
                        p2prev, pre = mid_out
                        if tcc < c.TC - 1:
                            xts_next = xload(tcc + 1)
                        ffs = phase2b(tcc - 1, *p2prev, pre)
                        if tcc < c.TC - 1:
                            hT_cur = phase1a_ln(tcc + 1, xts_next)
                        p2_out(tcc - 1, p2prev[0], ffs)
                        if tcc == c.TC - 1:
                            p2last = phase2a_pre(tcc)
                pre = prefetch_p2()
                ffs = phase2b(c.TC - 1, *p2last, pre)
                p2_out(c.TC - 1, p2last[0], ffs)

    nc.compile()
    return nc


def make_masks(cfg):
    c = cfg
    m = np.zeros((128, 2 * c.KTPQ * c.QCW), dtype=np.float32)
    for j in range(c.KTPQ):
        k = np.arange(128)[:, None]
        q = np.arange(c.QCW)[None, :]
        mj = (j * 128 + k <= q).astype(np.float32)
        m[:, 2 * j * c.QCW:(2 * j + 1) * c.QCW] = mj
        m[:, (2 * j + 1) * c.QCW:(2 * j + 2) * c.QCW] = mj
    return np.ascontiguousarray(m.astype(ml_dtypes.float8_e4m3))


def _part128(w):
    """[K, M] -> [128, (K//128) * M]: row p holds blocks k=e*128+p."""
    K, M = w.shape
    return np.ascontiguousarray(
        w.reshape(K // 128, 128, M).transpose(1, 0, 2).reshape(128, -1)
    )


def make_in_maps(cfg, inputs):
    """Build the per-core input dicts from the full problem inputs."""
    c = cfg
    x = np.asarray(inputs["x"], dtype=np.float32)
    ln1 = np.asarray(inputs["ln1_w"], dtype=np.float32)
    ln2 = np.asarray(inputs["ln2_w"], dtype=np.float32)
    # fold LN scale vectors into the consumer weight rows
    Wq = (ln1[:, None] * np.asarray(inputs["Wq"], dtype=np.float32)
          * (1.0 / np.sqrt(c.E)))
    Wk = ln1[:, None] * np.asarray(inputs["Wk"], dtype=np.float32)
    Wv = ln1[:, None] * np.asarray(inputs["Wv"], dtype=np.float32)
    W1 = ln2[:, None] * np.asarray(inputs["W1"], dtype=np.float32)
    Wo = np.asarray(inputs["Wo"], dtype=np.float32)
    W2 = np.asarray(inputs["W2"], dtype=np.float32)
    bo = np.asarray(inputs["bo"], dtype=np.float32)
    b1 = np.asarray(inputs["b1"], dtype=np.float32)
    b2 = np.asarray(inputs["b2"], dtype=np.float32)

    bf = ml_dtypes.bfloat16

    def rep(v):
        return np.ascontiguousarray(
            np.broadcast_to(v[None, :], (128, c.E)).astype(np.float32)
        )

    # full FFN weights (bf16), shared by the whole pair
    # w1 layout: [(FC//2)*128, EC*256]; row fgp*128+p, col e*256+m =
    # W1[e*128+p, fgp*256+m]
    w1p = W1.astype(bf).reshape(c.EC, 128, c.FC // 2, 256)
    w1p = np.ascontiguousarray(
        w1p.transpose(2, 1, 0, 3).reshape((c.FC // 2) * 128, c.EC * 256)
    )
    w2p = _part128(W2.astype(bf))  # [128, FC*E]
    b1p = np.ascontiguousarray(b1.reshape(c.FC, 128).T.astype(np.float32))

    consts = {
        "bor": rep(bo), "b2r": rep(b2),
        "masks": make_masks(c),
        "ident": np.eye(128, dtype=ml_dtypes.bfloat16),
        "w1": w1p, "w2": w2p, "b1": b1p,
    }
    in_maps = []
    for core in range(c.n_cores):
        b, g = core // 2, core % 2
        d0, d1 = g * c.D, (g + 1) * c.D
        xb = x[b]
        # own rows: chunk cc contributes rows [cc*512 + g*256, +256)
        xo = np.ascontiguousarray(
            xb.reshape(c.TC, 2, c.OWN, c.E)[:, g].reshape(c.T // 2, c.E)
        )
        def pair_major(w):  # [E, D] -> [128, PAIRS*E], pair-major cols
            w4 = w.reshape(c.EC, 128, c.PAIRS, 128)
            return np.ascontiguousarray(
                w4.transpose(1, 2, 0, 3).reshape(128, c.PAIRS * c.E)
            )

        fp8 = ml_dtypes.float8_e4m3
        m = {
            "x": np.ascontiguousarray(xb),
            "xo": xo,
            # x256 / x64 scaling keeps the 0.02-sigma weights out of the
            # fp8 subnormal range; rescaled in the PSUM->SBUF copies
            "wq": pair_major((256.0 * Wq[:, d0:d1]).astype(fp8)),
            "wk": pair_major((256.0 * Wk[:, d0:d1]).astype(fp8)),
            "wv": _part128((64.0 * Wv[:, d0:d1]).astype(fp8)),
            "wo": _part128(Wo[d0:d1, :].astype(bf)),
        }
        m.update(consts)
        in_maps.append(m)
    return in_maps


_NC_CACHE = {}


def get_nc(cfg):
    key = (cfg.B, cfg.T, cfg.E, cfg.HPC, cfg.FH, cfg.n_cores)
    if key not in _NC_CACHE:
        _NC_CACHE[key] = build_nc(cfg)
    return _NC_CACHE[key]


def assemble_out(cfg, results):
    c = cfg
    out = np.empty((c.B, c.T, c.E), dtype=np.float32)
    for b in range(c.B):
        for g in range(2):
            o = np.asarray(results[2 * b + g]["out"]).reshape(
                c.TC, c.OWN, c.E
            )
            out[b].reshape(c.TC, 2, c.OWN, c.E)[:, g] = o
    return out


def kernel(**inputs) -> np.ndarray:
    c = FULL
    nc = get_nc(c)
    in_maps = make_in_maps(c, inputs)
    res = run_bass_kernel_spmd(nc, in_maps, core_ids=list(range(c.n_cores)))
    return assemble_out(c, res.results)
